# revision 5
# baseline (speedup 1.0000x reference)
"""Trainium2 Bass kernel for nn_ConvUnit (bit-plane int8 conv, collapsed).

v6 (= v4 with warmup trimmed to 7 dummies: 5 ends too early and the
PE de-ramps before the first real matmul; 10 overruns the data-ready
point and delays it).  v4 over v3: weights host-packed partition-major so the weight DMA is
contiguous (the rearranged view produced 232B packets at 157GB/s and
gated the first matmul), split pairs-first/solos-second so the P phase
can start ~1us earlier, and a short stream of dummy matmuls warms the
PE p-state during the input-DMA wait (cold-start matmuls run ~2x slow
for the first ~3us otherwise).

See kernel_v2.py docstring for the math and layout.
"""

import numpy as np
import ml_dtypes

N_CORES = 8
N_IMG = 64
C_IN = 64
C_OUT = 128
H = W = 56
OH = OW = 54
IMGS_PER_CORE = N_IMG // N_CORES
R = H // 2  # 28 rows per parity

_cache = {}


def _build():
    import concourse.bass as bass
    import concourse.tile as tile
    from concourse import bacc, mybir

    nc = bacc.Bacc(None, target_bir_lowering=False, debug=False)
    dt = mybir.dt

    xp = nc.dram_tensor("xp", [IMGS_PER_CORE, 128, R, W], dt.int8,
                        kind="ExternalInput")
    # host-packed [partition, slot, m]: slots 0-5 = pair lhsTs
    # (even kw0-2, odd kw0-2), slots 6-11 = solo lhsTs
    wpk = nc.dram_tensor("wpk", [128, 12, 128], dt.bfloat16,
                         kind="ExternalInput")
    bias2 = nc.dram_tensor("bias2", [C_OUT, 1], dt.float32,
                           kind="ExternalInput")
    y = nc.dram_tensor("y", [IMGS_PER_CORE, C_OUT, OH, OW], dt.float16,
                       kind="ExternalOutput")

    with tile.TileContext(nc) as tc:
        with (
            tc.tile_pool(name="wpool", bufs=1) as wpool,
            tc.tile_pool(name="x8p", bufs=3) as x8p,
            tc.tile_pool(name="xq", bufs=3) as xqp,
            tc.tile_pool(name="psum", bufs=8, space=bass.MemorySpace.PSUM) as psp,
            tc.tile_pool(name="outp", bufs=2) as outp,
        ):
            # weights first (pair slots lead): the first matmul's
            # LDWEIGHTS gates on the first of these
            wsb = wpool.tile([128, 12, 128], dt.bfloat16)
            nc.scalar.dma_start(wsb[:, 0:3], wpk[:, 0:3])
            nc.scalar.dma_start(wsb[:, 3:6], wpk[:, 3:6])
            nc.scalar.dma_start(wsb[:, 6:12], wpk[:, 6:12])
            bsb = wpool.tile([C_OUT, 1], dt.float32)
            nc.scalar.dma_start(bsb[:], bias2[:])

            # PE p-state warmup: dummy matmuls on a zeroed scratch tile
            # while the first image's input DMA is in flight
            scr = wpool.tile([128, 540], dt.bfloat16)
            nc.vector.memset(scr[:], 0.0)
            wps = psp.tile([C_OUT, 486], dt.float32, tag="ps", name="warm")
            for _ in range(7):
                nc.tensor.matmul(wps[:], scr[:, 0:128], scr[:, 54:540],
                                 start=True, stop=True)

            for n in range(IMGS_PER_CORE):
                x8 = x8p.tile([128, R, W], dt.int8, tag="x8")
                xq = xqp.tile([128, R, W], dt.bfloat16, tag="xq")
                # half-image DMA + cast so rows 0..13 are ready mid-transfer
                for r0_, r1_ in ((0, 14), (14, R)):
                    nc.sync.dma_start(x8[:, r0_:r1_, :], xp[n][:, r0_:r1_, :])
                    nc.vector.tensor_copy(xq[:, r0_:r1_, :],
                                          x8[:, r0_:r1_, :])

                stage = outp.tile([C_OUT, OH, OW], dt.float16, tag="stage")
                stg = stage[:].rearrange("p (h2 q) w -> p h2 q w", q=2)

                # P phase: 18 K=128 pair matmuls, 6 open PSUM groups
                pss = {}
                for b in range(3):
                    r0 = 9 * b
                    for pi in range(2):
                        ps = psp.tile([C_OUT, 9, OW], dt.float32, tag="ps",
                                      name=f"ps_{n}_{b}_{pi}")
                        pss[(b, pi)] = ps
                        for kw in range(3):
                            # even: (kh0@par0, kh1@par1) at r0; odd:
                            # (kh1@par0, kh2@par1) at r0+1
                            slot = kw if pi == 0 else 3 + kw
                            roff = pi
                            nc.tensor.matmul(
                                ps[:], wsb[:, slot, :],
                                xq[:, r0 + roff:r0 + roff + 9, kw:kw + 54],
                                start=(kw == 0), stop=False)

                # S phase: 18 K=64 solos (run pairwise-concurrently on
                # complementary PE halves), closing each group
                for b in range(3):
                    r0 = 9 * b
                    for pi in range(2):
                        ps = pss[(b, pi)]
                        for kw in range(3):
                            if pi == 0:
                                # even solo: kh2 @ par0, rows r0+1
                                nc.tensor.matmul(
                                    ps[:], wsb[0:64, 6 + kw, :],
                                    xq[0:64, r0 + 1:r0 + 10, kw:kw + 54],
                                    start=False, stop=(kw == 2))
                            else:
                                # odd solo: kh0 @ par1, rows r0
                                nc.tensor.matmul(
                                    ps[:], wsb[64:128, 9 + kw, :],
                                    xq[64:128, r0:r0 + 9, kw:kw + 54],
                                    start=False, stop=(kw == 2))
                    nc.scalar.activation(
                        stg[:, r0:r0 + 9, 0, :], pss[(b, 0)][:],
                        mybir.ActivationFunctionType.Identity,
                        bias=bsb[:], scale=1.0)
                    if n == IMGS_PER_CORE - 1 and b == 2:
                        nc.vector.tensor_scalar(
                            stg[:, r0:r0 + 5, 1, :], pss[(b, 1)][:, 0:5, :],
                            bsb[:], None, mybir.AluOpType.add)
                        nc.scalar.activation(
                            stg[:, r0 + 5:r0 + 9, 1, :],
                            pss[(b, 1)][:, 5:9, :],
                            mybir.ActivationFunctionType.Identity,
                            bias=bsb[:], scale=1.0)
                    else:
                        nc.vector.tensor_scalar(
                            stg[:, r0:r0 + 9, 1, :], pss[(b, 1)][:], bsb[:],
                            None, mybir.AluOpType.add)
                    nc.sync.dma_start(y[n][:, 18 * b:18 * b + 18, :],
                                      stage[:, 18 * b:18 * b + 18, :])

    nc.compile()
    return nc


def _pack_weights(weight):
    # lhsT layouts: [K(c_in, possibly x2 parity), M(c_out)] per matmul
    # slot, stored partition-major [128, slot, 128] so the DMA is
    # contiguous.  Slots 0-2 even pairs, 3-5 odd pairs, 6-8 even solos,
    # 9-11 odd solos.
    wT = np.ascontiguousarray(weight.transpose(1, 0, 2, 3))  # [c_in,c_out,kh,kw]
    wpk = np.zeros((128, 12, 128), dtype=np.float32)
    for kw in range(3):
        wpk[0:64, kw] = wT[:, :, 0, kw]           # even pair: kh0 @ par0
        wpk[64:128, kw] = wT[:, :, 1, kw]         #            kh1 @ par1
        wpk[0:64, 3 + kw] = wT[:, :, 1, kw]       # odd pair:  kh1 @ par0
        wpk[64:128, 3 + kw] = wT[:, :, 2, kw]     #            kh2 @ par1
        wpk[0:64, 6 + kw] = wT[:, :, 2, kw]       # even solo: kh2 @ par0
        wpk[64:128, 9 + kw] = wT[:, :, 0, kw]     # odd solo:  kh0 @ par1
    return np.ascontiguousarray(wpk.astype(ml_dtypes.bfloat16))


def _pack_inputs(x):
    xi = np.clip(x, -128.0, 127.0).astype(np.int8)
    xp = np.empty((N_IMG, 128, R, W), np.int8)
    xp[:, 0:64] = xi[:, :, 0::2, :]
    xp[:, 64:128] = xi[:, :, 1::2, :]
    return np.ascontiguousarray(xp)


def kernel(x, weight, bias, _trace=False):
    from concourse.bass_utils import run_bass_kernel_spmd

    if "nc" not in _cache:
        _cache["nc"] = _build()
    nc = _cache["nc"]

    xp = _pack_inputs(np.asarray(x, dtype=np.float32))
    wpk = _pack_weights(np.asarray(weight, dtype=np.float32))
    b2 = np.ascontiguousarray(
        np.asarray(bias, dtype=np.float32).reshape(C_OUT, 1))

    in_maps = [
        {"xp": xp[i * IMGS_PER_CORE:(i + 1) * IMGS_PER_CORE], "wpk": wpk,
         "bias2": b2}
        for i in range(N_CORES)
    ]
    res = run_bass_kernel_spmd(nc, in_maps, list(range(N_CORES)),
                               trace=_trace)
    out = np.concatenate(
        [res.results[i]["y"].astype(np.float32) for i in range(N_CORES)],
        axis=0)
    if _trace:
        return out, res
    return out


# revision 6
# speedup vs baseline: 1.0222x; 1.0222x over previous
"""Trainium2 Bass kernel for nn_ConvUnit (bit-plane int8 conv, collapsed).

v6 (= v4 with warmup trimmed to 7 dummies: 5 ends too early and the
PE de-ramps before the first real matmul; 10 overruns the data-ready
point and delays it).  v4 over v3: weights host-packed partition-major so the weight DMA is
contiguous (the rearranged view produced 232B packets at 157GB/s and
gated the first matmul), split pairs-first/solos-second so the P phase
can start ~1us earlier, and a short stream of dummy matmuls warms the
PE p-state during the input-DMA wait (cold-start matmuls run ~2x slow
for the first ~3us otherwise).

See kernel_v2.py docstring for the math and layout.
"""

import numpy as np
import ml_dtypes

N_CORES = 8
N_IMG = 64
C_IN = 64
C_OUT = 128
H = W = 56
OH = OW = 54
IMGS_PER_CORE = N_IMG // N_CORES
R = H // 2  # 28 rows per parity

_cache = {}


def _build():
    import concourse.bass as bass
    import concourse.tile as tile
    from concourse import bacc, mybir

    nc = bacc.Bacc(None, target_bir_lowering=False, debug=False)
    dt = mybir.dt

    xp = nc.dram_tensor("xp", [IMGS_PER_CORE, 128, R, W], dt.int8,
                        kind="ExternalInput")
    # host-packed [partition, slot, m]: slots 0-5 = pair lhsTs
    # (even kw0-2, odd kw0-2), slots 6-11 = solo lhsTs
    wpk = nc.dram_tensor("wpk", [128, 12, 128], dt.bfloat16,
                         kind="ExternalInput")
    bias2 = nc.dram_tensor("bias2", [C_OUT, 1], dt.float32,
                           kind="ExternalInput")
    y = nc.dram_tensor("y", [IMGS_PER_CORE, C_OUT, OH, OW], dt.float16,
                       kind="ExternalOutput")

    with tile.TileContext(nc) as tc:
        with (
            tc.tile_pool(name="wpool", bufs=1) as wpool,
            tc.tile_pool(name="x8p", bufs=3) as x8p,
            tc.tile_pool(name="xq", bufs=3) as xqp,
            tc.tile_pool(name="psum", bufs=8, space=bass.MemorySpace.PSUM) as psp,
            tc.tile_pool(name="outp", bufs=2) as outp,
        ):
            # weights first (pair slots lead): the first matmul's
            # LDWEIGHTS gates on the first of these
            wsb = wpool.tile([128, 12, 128], dt.bfloat16)
            nc.scalar.dma_start(wsb[:, 0:3], wpk[:, 0:3])
            nc.scalar.dma_start(wsb[:, 3:6], wpk[:, 3:6])
            nc.scalar.dma_start(wsb[:, 6:12], wpk[:, 6:12])
            bsb = wpool.tile([C_OUT, 1], dt.float32)
            nc.scalar.dma_start(bsb[:], bias2[:])

            # PE p-state warmup: dummy matmuls on a zeroed scratch tile
            # while the first image's input DMA is in flight
            scr = wpool.tile([128, 540], dt.bfloat16)
            nc.gpsimd.memset(scr[:], 0.0)
            wps = psp.tile([C_OUT, 486], dt.float32, tag="ps", name="warm")
            for _ in range(8):
                nc.tensor.matmul(wps[:], scr[:, 0:128], scr[:, 54:540],
                                 start=True, stop=True)

            for n in range(IMGS_PER_CORE):
                x8 = x8p.tile([128, R, W], dt.int8, tag="x8")
                xq = xqp.tile([128, R, W], dt.bfloat16, tag="xq")
                # half-image DMA + cast so rows 0..13 are ready mid-transfer
                for r0_, r1_ in ((0, 14), (14, R)):
                    nc.sync.dma_start(x8[:, r0_:r1_, :], xp[n][:, r0_:r1_, :])
                    nc.vector.tensor_copy(xq[:, r0_:r1_, :],
                                          x8[:, r0_:r1_, :])

                stage = outp.tile([C_OUT, OH, OW], dt.float16, tag="stage")
                stg = stage[:].rearrange("p (h2 q) w -> p h2 q w", q=2)

                # P phase: 18 K=128 pair matmuls, 6 open PSUM groups
                pss = {}
                for b in range(3):
                    r0 = 9 * b
                    for pi in range(2):
                        ps = psp.tile([C_OUT, 9, OW], dt.float32, tag="ps",
                                      name=f"ps_{n}_{b}_{pi}")
                        pss[(b, pi)] = ps
                        for kw in range(3):
                            # even: (kh0@par0, kh1@par1) at r0; odd:
                            # (kh1@par0, kh2@par1) at r0+1
                            slot = kw if pi == 0 else 3 + kw
                            roff = pi
                            nc.tensor.matmul(
                                ps[:], wsb[:, slot, :],
                                xq[:, r0 + roff:r0 + roff + 9, kw:kw + 54],
                                start=(kw == 0), stop=False)

                # S phase: 18 K=64 solos (run pairwise-concurrently on
                # complementary PE halves), closing each group
                for b in range(3):
                    r0 = 9 * b
                    for pi in range(2):
                        ps = pss[(b, pi)]
                        for kw in range(3):
                            if pi == 0:
                                # even solo: kh2 @ par0, rows r0+1
                                nc.tensor.matmul(
                                    ps[:], wsb[0:64, 6 + kw, :],
                                    xq[0:64, r0 + 1:r0 + 10, kw:kw + 54],
                                    start=False, stop=(kw == 2))
                            else:
                                # odd solo: kh0 @ par1, rows r0
                                nc.tensor.matmul(
                                    ps[:], wsb[64:128, 9 + kw, :],
                                    xq[64:128, r0:r0 + 9, kw:kw + 54],
                                    start=False, stop=(kw == 2))
                    nc.scalar.activation(
                        stg[:, r0:r0 + 9, 0, :], pss[(b, 0)][:],
                        mybir.ActivationFunctionType.Identity,
                        bias=bsb[:], scale=1.0)
                    if n == IMGS_PER_CORE - 1 and b == 2:
                        nc.vector.tensor_scalar(
                            stg[:, r0:r0 + 5, 1, :], pss[(b, 1)][:, 0:5, :],
                            bsb[:], None, mybir.AluOpType.add)
                        nc.sync.dma_start(y[n][:, 36:46, :],
                                          stage[:, 36:46, :])
                        nc.scalar.activation(
                            stg[:, r0 + 5:r0 + 9, 1, :],
                            pss[(b, 1)][:, 5:9, :],
                            mybir.ActivationFunctionType.Identity,
                            bias=bsb[:], scale=1.0)
                        nc.sync.dma_start(y[n][:, 46:54, :],
                                          stage[:, 46:54, :])
                    else:
                        nc.vector.tensor_scalar(
                            stg[:, r0:r0 + 9, 1, :], pss[(b, 1)][:], bsb[:],
                            None, mybir.AluOpType.add)
                        nc.sync.dma_start(y[n][:, 18 * b:18 * b + 18, :],
                                          stage[:, 18 * b:18 * b + 18, :])

    nc.compile()
    return nc


def _pack_weights(weight):
    # lhsT layouts: [K(c_in, possibly x2 parity), M(c_out)] per matmul
    # slot, stored partition-major [128, slot, 128] so the DMA is
    # contiguous.  Slots 0-2 even pairs, 3-5 odd pairs, 6-8 even solos,
    # 9-11 odd solos.
    wT = np.ascontiguousarray(weight.transpose(1, 0, 2, 3))  # [c_in,c_out,kh,kw]
    wpk = np.zeros((128, 12, 128), dtype=np.float32)
    for kw in range(3):
        wpk[0:64, kw] = wT[:, :, 0, kw]           # even pair: kh0 @ par0
        wpk[64:128, kw] = wT[:, :, 1, kw]         #            kh1 @ par1
        wpk[0:64, 3 + kw] = wT[:, :, 1, kw]       # odd pair:  kh1 @ par0
        wpk[64:128, 3 + kw] = wT[:, :, 2, kw]     #            kh2 @ par1
        wpk[0:64, 6 + kw] = wT[:, :, 2, kw]       # even solo: kh2 @ par0
        wpk[64:128, 9 + kw] = wT[:, :, 0, kw]     # odd solo:  kh0 @ par1
    return np.ascontiguousarray(wpk.astype(ml_dtypes.bfloat16))


def _pack_inputs(x):
    xi = np.clip(x, -128.0, 127.0).astype(np.int8)
    xp = np.empty((N_IMG, 128, R, W), np.int8)
    xp[:, 0:64] = xi[:, :, 0::2, :]
    xp[:, 64:128] = xi[:, :, 1::2, :]
    return np.ascontiguousarray(xp)


def kernel(x, weight, bias, _trace=False):
    from concourse.bass_utils import run_bass_kernel_spmd

    if "nc" not in _cache:
        _cache["nc"] = _build()
    nc = _cache["nc"]

    xp = _pack_inputs(np.asarray(x, dtype=np.float32))
    wpk = _pack_weights(np.asarray(weight, dtype=np.float32))
    b2 = np.ascontiguousarray(
        np.asarray(bias, dtype=np.float32).reshape(C_OUT, 1))

    in_maps = [
        {"xp": xp[i * IMGS_PER_CORE:(i + 1) * IMGS_PER_CORE], "wpk": wpk,
         "bias2": b2}
        for i in range(N_CORES)
    ]
    res = run_bass_kernel_spmd(nc, in_maps, list(range(N_CORES)),
                               trace=_trace)
    out = np.concatenate(
        [res.results[i]["y"].astype(np.float32) for i in range(N_CORES)],
        axis=0)
    if _trace:
        return out, res
    return out
